# revision 9
# baseline (speedup 1.0000x reference)
"""Causal self-attention (B=4, T=2048, C=1024, H=16, D=64) on 8 TRN2 NeuronCores.

Sharding: core c handles batch b=c//2 and head-group g=c%2 (8 of 16 heads,
i.e. 512 of 1024 feature columns). Each core:
  - projects q,k,v for its heads from x[b]  (contractions over C)
  - computes causal softmax(q k^T / sqrt(d)) v for its 8 heads
  - computes the partial o_proj  attn_out[:, cols] @ Wo[:, cols].T  -> [T, C]
Host sums the two head-group partials per batch and stacks batches.

Device layouts (host pre-transposes so every matmul contracts over the
partition dim with unit-stride DMAs):
  xT  [C, T]    = x[b].T
  wqT/wkT/wvT [C, 512] = W.T[:, cols]
  woT [512, C] = Wo.T[cols, :]
Scores are computed transposed (S^T[tk, tq] per head) so the PV matmul needs
no transpose; softmax denominators come from a ones-column appended to V
(PV matmul has M=65, row 64 = sum of weights). Two heads are packed per
128-partition tile; their K=64 QK matmuls run row-packed at
tile_position (0,0)/(64,0), and both heads share one [128,1024] exp.
"""

import numpy as np

B, T, C, H, D = 4, 2048, 1024, 16, 64
NCORES = 8
FH = 512          # features per core = 8 heads
NCT = C // 128    # 8 contraction tiles
FT = 4            # head-pair tiles (8 heads / 2)
NQ = 4            # tq chunks of 512
CH = 512          # tq chunk width

_CACHE = {}


def _build():
    import concourse.bass as bass  # noqa: F401
    import concourse.mybir as mybir
    from concourse import bacc
    from concourse.tile import TileContext

    F32 = mybir.dt.float32
    BF16 = mybir.dt.bfloat16
    EXP = mybir.ActivationFunctionType.Exp

    nc = bacc.Bacc("TRN2", target_bir_lowering=False, debug=False, num_devices=NCORES)
    xT_h = nc.dram_tensor("xT", [C, T], F32, kind="ExternalInput")
    wq_h = nc.dram_tensor("wqT", [C, FH], F32, kind="ExternalInput")
    wk_h = nc.dram_tensor("wkT", [C, FH], F32, kind="ExternalInput")
    wv_h = nc.dram_tensor("wvT", [C, FH], F32, kind="ExternalInput")
    wo_h = nc.dram_tensor("woT", [FH, C], F32, kind="ExternalInput")
    out_h = nc.dram_tensor("out", [T, C], F32, kind="ExternalOutput")
    xT = xT_h.ap()
    out_ap = out_h.ap()

    with TileContext(nc) as tc:
        with (
            tc.tile_pool(name="persist", bufs=1) as persist,
            tc.tile_pool(name="xp", bufs=2) as xp,
            tc.tile_pool(name="qp", bufs=2) as qp,
            tc.tile_pool(name="ptp", bufs=2) as ptp,
            tc.tile_pool(name="apl", bufs=2) as apool,
            tc.tile_pool(name="opl", bufs=2) as opool,
            tc.tile_pool(name="rp", bufs=4) as rp,
            tc.tile_pool(name="rbp", bufs=2) as rbp,
            tc.tile_pool(name="dp", bufs=4, space="DRAM") as dp,
            tc.tile_pool(name="pp", bufs=2, space="PSUM") as pp,
            tc.tile_pool(name="sp", bufs=2, space="PSUM") as sp,
            tc.tile_pool(name="vp", bufs=2, space="PSUM") as vp,
        ):
            wq_s = persist.tile([128, NCT, FH], F32, tag="wq")
            wk_s = persist.tile([128, NCT, FH], F32, tag="wk")
            wv_s = persist.tile([128, NCT, FH], F32, tag="wv")
            wo_s = persist.tile([128, FT, C], F32, tag="wo")
            nc.sync.dma_start(out=wq_s, in_=wq_h.ap().rearrange("(c p) f -> p c f", p=128))
            nc.sync.dma_start(out=wk_s, in_=wk_h.ap().rearrange("(c p) f -> p c f", p=128))
            nc.sync.dma_start(out=wv_s, in_=wv_h.ap().rearrange("(c p) f -> p c f", p=128))
            nc.sync.dma_start(out=wo_s, in_=wo_h.ap().rearrange("(i p) f -> p i f", p=128))

            # causal 0/1 mask: mb[p, g] = 1 iff g - 384 >= p.
            # diagonal tile r uses slice mb[:, 384-128r : 384-128r+512]:
            # value 1 iff f >= 128*r + p  (f = tq offset in chunk, p = tk offset in tile)
            mb = persist.tile([128, 896], BF16, tag="mb")
            nc.gpsimd.memset(mb, 1.0)
            nc.gpsimd.affine_select(
                out=mb, in_=mb, compare_op=mybir.AluOpType.is_ge, fill=0.0,
                base=-384, pattern=[[1, 896]], channel_multiplier=-1,
            )

            kT_s = persist.tile([128, FT, T], F32, tag="kT")
            v_s = persist.tile([128, 16, 8, 65], BF16, tag="vs")
            nc.gpsimd.memset(v_s[:, :, :, 64:65], 1.0)

            # ---- phase A: k^T and v projections, streamed over t-chunks ----
            for q in range(NQ):
                xt = xp.tile([128, NCT, CH], F32, tag="xt")
                nc.sync.dma_start(
                    out=xt,
                    in_=xT[:, q * CH:(q + 1) * CH].rearrange("(c p) t -> p c t", p=128),
                )
                for j in range(FT):
                    ps = pp.tile([128, 512], F32, tag="pp")
                    for c in range(NCT):
                        nc.tensor.matmul(
                            ps, wk_s[:, c, j * 128:(j + 1) * 128], xt[:, c, :],
                            start=(c == 0), stop=(c == NCT - 1),
                        )
                    nc.vector.tensor_copy(out=kT_s[:, j, q * CH:(q + 1) * CH], in_=ps)
                for tt in range(4):
                    i = q * 4 + tt
                    ps = pp.tile([128, 512], F32, tag="pp")
                    for c in range(NCT):
                        nc.tensor.matmul(
                            ps, xt[:, c, tt * 128:(tt + 1) * 128], wv_s[:, c, :],
                            start=(c == 0), stop=(c == NCT - 1),
                        )
                    nc.vector.tensor_copy(
                        out=v_s[:, i, :, 0:64], in_=ps.rearrange("p (h d) -> p h d", h=8)
                    )

            # ---- phase B: per tq chunk: q^T, attention, o_proj partial ----
            for q in range(NQ):
                xt = xp.tile([128, NCT, CH], F32, tag="xt")
                nc.sync.dma_start(
                    out=xt,
                    in_=xT[:, q * CH:(q + 1) * CH].rearrange("(c p) t -> p c t", p=128),
                )
                qT = qp.tile([128, FT, CH], F32, tag="qT")
                for j in range(FT):
                    ps = pp.tile([128, 512], F32, tag="pp")
                    for c in range(NCT):
                        nc.tensor.matmul(
                            ps, wq_s[:, c, j * 128:(j + 1) * 128], xt[:, c, :],
                            start=(c == 0), stop=(c == NCT - 1),
                        )
                    nc.vector.tensor_copy(out=qT[:, j, :], in_=ps)

                attnT = apool.tile([128, FT, CH], F32, tag="attnT")
                nkt = 4 * q + 4
                for j in range(FT):
                    pvA = vp.tile([65, 512], F32, tag="pv")
                    pvB = vp.tile([65, 512], F32, tag="pv")
                    for kt in range(nkt):
                        s_ps = sp.tile([128, 1024], F32, tag="s")
                        nc.tensor.matmul(
                            s_ps[:, 0:512], kT_s[0:64, j, kt * 128:(kt + 1) * 128],
                            qT[0:64, j, :], start=True, stop=True, tile_position=(0, 0),
                        )
                        nc.tensor.matmul(
                            s_ps[:, 512:1024], kT_s[64:128, j, kt * 128:(kt + 1) * 128],
                            qT[64:128, j, :], start=True, stop=True, tile_position=(64, 0),
                        )
                        pt = ptp.tile([128, 1024], BF16, tag="pt")
                        nc.scalar.activation(out=pt, in_=s_ps, func=EXP, scale=0.125)
                        if kt >= 4 * q:
                            r = kt - 4 * q
                            msl = mb[:, 384 - 128 * r: 384 - 128 * r + 512]
                            nc.vector.tensor_mul(pt[:, 0:512], pt[:, 0:512], msl)
                            nc.vector.tensor_mul(pt[:, 512:1024], pt[:, 512:1024], msl)
                        nc.tensor.matmul(
                            pvA, v_s[:, kt, 2 * j, :], pt[:, 0:512],
                            start=(kt == 0), stop=(kt == nkt - 1), skip_group_check=True,
                        )
                        nc.tensor.matmul(
                            pvB, v_s[:, kt, 2 * j + 1, :], pt[:, 512:1024],
                            start=(kt == 0), stop=(kt == nkt - 1), skip_group_check=True,
                        )
                    # softmax denominators: 1/sum per head, broadcast across 64
                    # partitions via a DRAM round-trip (SBUF APs cannot have
                    # zero partition step; DRAM sources can).
                    rq = rp.tile([1, 1024], F32, tag="rec")
                    nc.vector.reciprocal(out=rq[0:1, 0:512], in_=pvA[64:65, :])
                    nc.vector.reciprocal(out=rq[0:1, 512:1024], in_=pvB[64:65, :])
                    rd = dp.tile([1, 1024], F32, tag="rd")
                    nc.sync.dma_start(out=rd, in_=rq)
                    rbc = rbp.tile([128, 512], F32, tag="rbc")
                    nc.sync.dma_start(out=rbc[0:64, :], in_=rd[0:1, 0:512].broadcast_to([64, 512]))
                    nc.sync.dma_start(out=rbc[64:128, :], in_=rd[0:1, 512:1024].broadcast_to([64, 512]))
                    nc.vector.tensor_mul(attnT[0:64, j, :], pvA[0:64, :], rbc[0:64, :])
                    nc.vector.tensor_mul(attnT[64:128, j, :], pvB[0:64, :], rbc[64:128, :])

                for n in range(2):
                    for mt in range(4):
                        po = pp.tile([128, 512], F32, tag="pp")
                        for i in range(FT):
                            nc.tensor.matmul(
                                po, attnT[:, i, mt * 128:(mt + 1) * 128],
                                wo_s[:, i, n * 512:(n + 1) * 512],
                                start=(i == 0), stop=(i == FT - 1),
                            )
                        ot = opool.tile([128, 512], F32, tag="ot")
                        nc.vector.tensor_copy(out=ot, in_=po)
                        nc.sync.dma_start(
                            out=out_ap[q * CH + mt * 128: q * CH + (mt + 1) * 128,
                                       n * 512:(n + 1) * 512],
                            in_=ot,
                        )

    nc.compile()
    return nc


def _get_nc():
    if "nc" not in _CACHE:
        _CACHE["nc"] = _build()
    return _CACHE["nc"]


def make_in_maps(x, Wq, Wk, Wv, Wo):
    x = np.asarray(x, dtype=np.float32)
    Wq = np.asarray(Wq, dtype=np.float32)
    Wk = np.asarray(Wk, dtype=np.float32)
    Wv = np.asarray(Wv, dtype=np.float32)
    Wo = np.asarray(Wo, dtype=np.float32)
    in_maps = []
    for core in range(NCORES):
        b, g = core // 2, core % 2
        cols = slice(FH * g, FH * (g + 1))
        in_maps.append({
            "xT": np.ascontiguousarray(x[b].T),
            "wqT": np.ascontiguousarray(Wq.T[:, cols]),
            "wkT": np.ascontiguousarray(Wk.T[:, cols]),
            "wvT": np.ascontiguousarray(Wv.T[:, cols]),
            "woT": np.ascontiguousarray(Wo.T[cols, :]),
        })
    return in_maps


def gather_out(parts):
    return np.stack([parts[2 * b] + parts[2 * b + 1] for b in range(B)])


def kernel(x, Wq, Wk, Wv, Wo):
    from concourse.bass_utils import run_bass_kernel_spmd

    nc = _get_nc()
    in_maps = make_in_maps(x, Wq, Wk, Wv, Wo)
    res = run_bass_kernel_spmd(nc, in_maps, core_ids=list(range(NCORES)))
    return gather_out([res.results[c]["out"] for c in range(NCORES)])


# revision 20
# speedup vs baseline: 1.1242x; 1.1242x over previous
"""Causal self-attention (B=4, T=2048, C=1024, H=16, D=64) on 8 TRN2 NeuronCores.

Sharding: core c handles batch b=c//2 and head-group g=c%2 (8 of 16 heads,
i.e. 512 of 1024 feature columns). Each core:
  - projects q,k,v for its heads from x[b]  (contractions over C)
  - computes causal softmax(q k^T / sqrt(d)) v for its 8 heads
  - computes the partial o_proj  attn_out[:, cols] @ Wo[:, cols].T  -> [T, C]
Host sums the two head-group partials per batch and stacks batches.

Device layouts (host pre-transposes so every matmul contracts over the
partition dim with unit-stride DMAs):
  xT  [C, T]    = x[b].T
  wqT/wkT/wvT [C, 512] = W.T[:, cols]
  woT [512, C] = Wo.T[cols, :]
Scores are computed transposed (S^T[tk, tq] per head) so the PV matmul needs
no transpose; softmax denominators come from a ones-column appended to V
(PV matmul has M=65, row 64 = sum of weights). Two heads are packed per
128-partition tile; their K=64 QK matmuls run row-packed at
tile_position (0,0)/(64,0), and both heads share one [128,1024] exp.
"""

import numpy as np

B, T, C, H, D = 4, 2048, 1024, 16, 64
NCORES = 8
FH = 512          # features per core = 8 heads
NCT = C // 128    # 8 contraction tiles
FT = 4            # head-pair tiles (8 heads / 2)
NQ = 4            # tq chunks of 512
CH = 512          # tq chunk width

_CACHE = {}


def _build(reps=1):
    import concourse.bass as bass  # noqa: F401
    import concourse.mybir as mybir
    from concourse import bacc
    from concourse.tile import TileContext

    F32 = mybir.dt.float32
    F32R = mybir.dt.float32r
    BF16 = mybir.dt.bfloat16

    EXP = mybir.ActivationFunctionType.Exp

    nc = bacc.Bacc("TRN2", target_bir_lowering=False, debug=False, num_devices=NCORES)
    xT_h = nc.dram_tensor("xT", [C, T], F32R, kind="ExternalInput")
    wq_h = nc.dram_tensor("wqT", [C, FH], F32R, kind="ExternalInput")
    wk_h = nc.dram_tensor("wkT", [C, FH], F32R, kind="ExternalInput")
    wv_h = nc.dram_tensor("wvT", [C, FH], F32R, kind="ExternalInput")
    wo_h = nc.dram_tensor("woT", [FH, C], F32R, kind="ExternalInput")
    out_h = nc.dram_tensor("out", [T, C], F32, kind="ExternalOutput")
    xT = xT_h.ap()
    out_ap = out_h.ap()

    with TileContext(nc) as tc:
        with (
            tc.tile_pool(name="persist", bufs=1) as persist,
            tc.tile_pool(name="xp", bufs=2) as xp,
            tc.tile_pool(name="qp", bufs=2) as qp,
            tc.tile_pool(name="ptp", bufs=2) as ptp,
            tc.tile_pool(name="apl", bufs=2) as apool,
            tc.tile_pool(name="opl", bufs=2) as opool,
            tc.tile_pool(name="rp", bufs=4) as rp,
            tc.tile_pool(name="rbp", bufs=2) as rbp,
            tc.tile_pool(name="dp", bufs=4, space="DRAM") as dp,
            tc.tile_pool(name="pp", bufs=2, space="PSUM") as pp,
            tc.tile_pool(name="sp", bufs=2, space="PSUM") as sp,
            tc.tile_pool(name="vp", bufs=2, space="PSUM") as vp,
        ):
            wq_s = persist.tile([128, NCT, FH], F32R, tag="wq")
            wk_s = persist.tile([128, NCT, FH], F32R, tag="wk")
            wv_s = persist.tile([128, NCT, FH], F32R, tag="wv")
            wo_s = persist.tile([128, FT, C], F32R, tag="wo")
            wk_src = wk_h.ap().rearrange("(c p) f -> p c f", p=128)
            wv_src = wv_h.ap().rearrange("(c p) f -> p c f", p=128)
            xt0_pre = xp.tile([128, NCT, CH], F32R, tag="xt")
            xt0_src = xT[:, 0:CH].rearrange("(c p) t -> p c t", p=128)
            nc.sync.dma_start(out=wk_s[:, 0:4, :], in_=wk_src[:, 0:4, :])
            nc.sync.dma_start(out=xt0_pre[:, 0:4, :], in_=xt0_src[:, 0:4, :])
            nc.sync.dma_start(out=wk_s[:, 4:8, :], in_=wk_src[:, 4:8, :])
            nc.sync.dma_start(out=xt0_pre[:, 4:8, :], in_=xt0_src[:, 4:8, :])
            nc.sync.dma_start(out=wv_s[:, 0:4, :], in_=wv_src[:, 0:4, :])
            nc.sync.dma_start(out=wv_s[:, 4:8, :], in_=wv_src[:, 4:8, :])

            # causal 0/1 mask: mb[p, g] = 1 iff g - 384 >= p.
            # diagonal tile r uses slice mb[:, 384-128r : 384-128r+512]:
            # value 1 iff f >= 128*r + p  (f = tq offset in chunk, p = tk offset in tile)
            mb = persist.tile([128, 896], BF16, tag="mb")
            nc.gpsimd.memset(mb, 1.0)
            nc.gpsimd.affine_select(
                out=mb, in_=mb, compare_op=mybir.AluOpType.is_ge, fill=0.0,
                base=-384, pattern=[[1, 896]], channel_multiplier=-1,
            )

            kT_s = persist.tile([128, FT, T], F32R, tag="kT")
            v_s = persist.tile([128, 16, 8, 65], BF16, tag="vs")
            nc.gpsimd.memset(v_s[:, :, :, 64:65], 1.0)

            # ---- phase A: k^T and v projections, streamed over t-chunks ----
            SENT = object()
            for _rep in range(reps):
              for q in range(NQ):
                if q == 0 and _rep == 0:
                    xt = xt0_pre
                else:
                    xt = xp.tile([128, NCT, CH], F32R, tag="xt")
                    xt_src = xT[:, q * CH:(q + 1) * CH].rearrange("(c p) t -> p c t", p=128)
                    nc.sync.dma_start(out=xt[:, 0:4, :], in_=xt_src[:, 0:4, :])
                    nc.sync.dma_start(out=xt[:, 4:8, :], in_=xt_src[:, 4:8, :])
                for j in range(FT):
                    ps = pp.tile([128, 512], F32, tag="pp")
                    for c in range(NCT):
                        nc.tensor.matmul(
                            ps, wk_s[:, c, j * 128:(j + 1) * 128], xt[:, c, :],
                            start=(c == 0), stop=(c == NCT - 1),
                        )
                    nc.scalar.copy(out=kT_s[:, j, q * CH:(q + 1) * CH], in_=ps)
                for tt in range(4):
                    i = q * 4 + tt
                    ps = pp.tile([128, 512], F32, tag="pp")
                    for c in range(NCT):
                        nc.tensor.matmul(
                            ps, xt[:, c, tt * 128:(tt + 1) * 128], wv_s[:, c, :],
                            start=(c == 0), stop=(c == NCT - 1),
                        )
                    nc.vector.tensor_copy(
                        out=v_s[:, i, :, 0:64], in_=ps.rearrange("p (h d) -> p h d", h=8)
                    )
                if q == NQ - 1 and _rep == 0:
                    # q/o weights are first needed in phase B; keep them off the
                    # startup critical path
                    nc.sync.dma_start(out=wq_s, in_=wq_h.ap().rearrange("(c p) f -> p c f", p=128))
                    nc.sync.dma_start(out=wo_s, in_=wo_h.ap().rearrange("(i p) f -> p i f", p=128))

              # ---- phase B: per tq chunk: attention, with next chunk's q^T
              # projection and previous chunk's o_proj matmuls interleaved
              # into the exp-paced inner loop so the PE never starves ----
              def load_xt(lq):
                  t = xp.tile([128, NCT, CH], F32R, tag="xt")
                  src = xT[:, lq * CH:(lq + 1) * CH].rearrange("(c p) t -> p c t", p=128)
                  nc.sync.dma_start(out=t[:, 0:4, :], in_=src[:, 0:4, :])
                  nc.sync.dma_start(out=t[:, 4:8, :], in_=src[:, 4:8, :])
                  return t

              def qproj_steps(qT_t, xt_t):
                  for jj in range(FT):
                      ps = pp.tile([128, 512], F32, tag="pp")
                      for c in range(NCT):
                          nc.tensor.matmul(
                              ps, wq_s[:, c, jj * 128:(jj + 1) * 128], xt_t[:, c, :],
                              start=(c == 0), stop=(c == NCT - 1), skip_group_check=True,
                          )
                          yield
                      nc.vector.tensor_copy(out=qT_t[:, jj, :], in_=ps)
                      yield

              def oproj_steps(oq, at):
                  for n in range(2):
                      for mt in range(4):
                          po = pp.tile([128, 512], F32, tag="pp")
                          for i in range(FT):
                              nc.tensor.matmul(
                                  po, at[:, i, mt * 128:(mt + 1) * 128],
                                  wo_s[:, i, n * 512:(n + 1) * 512],
                                  start=(i == 0), stop=(i == FT - 1), skip_group_check=True,
                              )
                              yield
                          ot = opool.tile([128, 512], F32, tag="ot")
                          nc.vector.tensor_copy(out=ot, in_=po)
                          nc.sync.dma_start(
                              out=out_ap[oq * CH + mt * 128: oq * CH + (mt + 1) * 128,
                                         n * 512:(n + 1) * 512],
                              in_=ot,
                          )
                          yield

              def chain(*gens):
                  for gg in gens:
                      yield from gg

              prev = None
              xt_b = load_xt(0)
              qT_b = qp.tile([128, FT, CH], F32R, tag="qT")
              for _ in qproj_steps(qT_b, xt_b):
                  pass
              for q in range(NQ):
                  qT = qT_b
                  gens = []
                  n_steps = 0
                  if prev is not None:
                      gens.append(oproj_steps(*prev))
                      n_steps += 40
                  if q + 1 < NQ:
                      xt_b = load_xt(q + 1)
                      qT_b = qp.tile([128, FT, CH], F32R, tag="qT")
                      gens.append(qproj_steps(qT_b, xt_b))
                      n_steps += 36
                  stream = chain(*gens)
                  nkt = 4 * q + 4
                  # delay the stream when it starts with qproj (chunk 0): its
                  # xt DMA was only just issued
                  lead = 4 if prev is None else 0
                  total_kt = nkt * FT
                  emitted = 0
                  done_kt = 0
                  attnT = apool.tile([128, FT, CH], F32R, tag="attnT")
                  for j in range(FT):
                      pvA = vp.tile([65, 512], F32, tag="pv")
                      pvB = vp.tile([65, 512], F32, tag="pv")
                      def emit_pv(kkt, ptile):
                          nc.tensor.matmul(
                              pvA, v_s[:, kkt, 2 * j, :], ptile[:, 0:512],
                              start=(kkt == 0), stop=(kkt == nkt - 1), skip_group_check=True,
                          )
                          nc.tensor.matmul(
                              pvB, v_s[:, kkt, 2 * j + 1, :], ptile[:, 512:1024],
                              start=(kkt == 0), stop=(kkt == nkt - 1), skip_group_check=True,
                          )
                      pend = None
                      for kt in range(nkt):
                          s_ps = sp.tile([128, 1024], F32, tag="s")
                          nc.tensor.matmul(
                              s_ps[:, 0:512], kT_s[0:64, j, kt * 128:(kt + 1) * 128],
                              qT[0:64, j, :], start=True, stop=True, tile_position=(0, 0),
                          )
                          nc.tensor.matmul(
                              s_ps[:, 512:1024], kT_s[64:128, j, kt * 128:(kt + 1) * 128],
                              qT[64:128, j, :], start=True, stop=True, tile_position=(64, 0),
                          )
                          pt = ptp.tile([128, 1024], BF16, tag="pt")
                          nc.scalar.activation(out=pt, in_=s_ps, func=EXP, scale=0.125)
                          if kt >= 4 * q:
                              rr = kt - 4 * q
                              msl = mb[:, 384 - 128 * rr: 384 - 128 * rr + 512]
                              nc.vector.tensor_mul(pt[:, 0:512], pt[:, 0:512], msl)
                              nc.vector.tensor_mul(pt[:, 512:1024], pt[:, 512:1024], msl)
                          done_kt += 1
                          want = n_steps * max(0, done_kt - lead) // max(1, total_kt - lead)
                          while emitted < want:
                              if next(stream, SENT) is SENT:
                                  emitted = n_steps
                                  break
                              emitted += 1
                          if pend is not None:
                              emit_pv(*pend)
                          pend = (kt, pt)
                      emit_pv(*pend)
                      # Move PV results out unnormalized (releases the PSUM
                      # accumulators fast), then normalize attnT in place once
                      # the reciprocal row returns from its DRAM broadcast
                      # round-trip (SBUF APs cannot have zero partition step;
                      # DRAM sources can).
                      rq = rp.tile([1, 1024], F32, tag="rec")
                      nc.vector.reciprocal(out=rq[0:1, 0:512], in_=pvA[64:65, :])
                      nc.vector.reciprocal(out=rq[0:1, 512:1024], in_=pvB[64:65, :])
                      nc.vector.tensor_copy(out=attnT[0:64, j, :], in_=pvA[0:64, :])
                      nc.vector.tensor_copy(out=attnT[64:128, j, :], in_=pvB[0:64, :])
                      rd = dp.tile([1, 1024], F32, tag="rd")
                      nc.sync.dma_start(out=rd, in_=rq)
                      rbc = rbp.tile([128, 512], F32, tag="rbc")
                      nc.sync.dma_start(out=rbc[0:64, :], in_=rd[0:1, 0:512].broadcast_to([64, 512]))
                      nc.sync.dma_start(out=rbc[64:128, :], in_=rd[0:1, 512:1024].broadcast_to([64, 512]))
                      nc.vector.tensor_mul(attnT[0:64, j, :], attnT[0:64, j, :], rbc[0:64, :])
                      nc.vector.tensor_mul(attnT[64:128, j, :], attnT[64:128, j, :], rbc[64:128, :])
                  for _ in stream:
                      pass
                  prev = (q, attnT)
              oproj_steps_tail = oproj_steps(*prev)
              for _ in oproj_steps_tail:
                  pass

    nc.compile()
    return nc


def _get_nc():
    if "nc" not in _CACHE:
        _CACHE["nc"] = _build()
    return _CACHE["nc"]


def make_in_maps(x, Wq, Wk, Wv, Wo):
    x = np.asarray(x, dtype=np.float32)
    Wq = np.asarray(Wq, dtype=np.float32)
    Wk = np.asarray(Wk, dtype=np.float32)
    Wv = np.asarray(Wv, dtype=np.float32)
    Wo = np.asarray(Wo, dtype=np.float32)
    in_maps = []
    for core in range(NCORES):
        b, g = core // 2, core % 2
        cols = slice(FH * g, FH * (g + 1))
        in_maps.append({
            "xT": np.ascontiguousarray(x[b].T),
            "wqT": np.ascontiguousarray(Wq.T[:, cols]),
            "wkT": np.ascontiguousarray(Wk.T[:, cols]),
            "wvT": np.ascontiguousarray(Wv.T[:, cols]),
            "woT": np.ascontiguousarray(Wo.T[cols, :]),
        })
    return in_maps


def gather_out(parts):
    return np.stack([parts[2 * b] + parts[2 * b + 1] for b in range(B)])


def kernel(x, Wq, Wk, Wv, Wo):
    from concourse.bass_utils import run_bass_kernel_spmd

    nc = _get_nc()
    in_maps = make_in_maps(x, Wq, Wk, Wv, Wo)
    res = run_bass_kernel_spmd(nc, in_maps, core_ids=list(range(NCORES)))
    return gather_out([res.results[c]["out"] for c in range(NCORES)])


# revision 32
# speedup vs baseline: 65.6944x; 58.4349x over previous
"""Causal self-attention (B=4, T=2048, C=1024, H=16, D=64) on 8 TRN2 NeuronCores.

Sharding: core c handles batch b=c//2 and head-group g=c%2 (8 of 16 heads,
i.e. 512 of 1024 feature columns). Each core:
  - projects q,k,v for its heads from x[b]  (contractions over C)
  - computes causal softmax(q k^T / sqrt(d)) v for its 8 heads
  - computes the partial o_proj  attn_out[:, cols] @ Wo[:, cols].T  -> [T, C]
Host sums the two head-group partials per batch and stacks batches.

Device layouts (host pre-transposes so every matmul contracts over the
partition dim with unit-stride DMAs):
  xT  [C, T]    = x[b].T
  wqT/wkT/wvT [C, 512] = W.T[:, cols]
  woT [512, C] = Wo.T[cols, :]
Scores are computed transposed (S^T[tk, tq] per head) so the PV matmul needs
no transpose; softmax denominators come from a ones-column appended to V
(PV matmul has M=65, row 64 = sum of weights). Two heads are packed per
128-partition tile; their K=64 QK matmuls run row-packed at
tile_position (0,0)/(64,0), and both heads share one [128,1024] exp.
"""

import numpy as np

B, T, C, H, D = 4, 2048, 1024, 16, 64
NCORES = 8
FH = 512          # features per core = 8 heads
NCT = C // 128    # 8 contraction tiles
FT = 4            # head-pair tiles (8 heads / 2)
NQ = 4            # tq chunks of 512
CH = 512          # tq chunk width

_CACHE = {}


def _build(reps=1):
    import concourse.bass as bass  # noqa: F401
    import concourse.mybir as mybir
    from concourse import bacc
    from concourse.tile import TileContext

    F32 = mybir.dt.float32
    F32R = mybir.dt.float32r
    BF16 = mybir.dt.bfloat16

    EXP = mybir.ActivationFunctionType.Exp

    nc = bacc.Bacc("TRN2", target_bir_lowering=False, debug=False, num_devices=NCORES)
    xT_h = nc.dram_tensor("xT", [C, T], F32R, kind="ExternalInput")
    wq_h = nc.dram_tensor("wqT", [C, FH], F32R, kind="ExternalInput")
    wk_h = nc.dram_tensor("wkT", [C, FH], F32R, kind="ExternalInput")
    wv_h = nc.dram_tensor("wvT", [C, FH], F32R, kind="ExternalInput")
    wo_h = nc.dram_tensor("woT", [FH, C], F32R, kind="ExternalInput")
    out_h = nc.dram_tensor("out", [T, C], F32, kind="ExternalOutput")
    xT = xT_h.ap()
    out_ap = out_h.ap()

    with TileContext(nc) as tc:
        with (
            tc.tile_pool(name="persist", bufs=1) as persist,
            tc.tile_pool(name="xp", bufs=2) as xp,
            tc.tile_pool(name="qp", bufs=2) as qp,
            tc.tile_pool(name="ptp", bufs=2) as ptp,
            tc.tile_pool(name="apl", bufs=2) as apool,
            tc.tile_pool(name="opl", bufs=2) as opool,
            tc.tile_pool(name="rp", bufs=2) as rp,
            tc.tile_pool(name="rbp", bufs=2) as rbp,
            tc.tile_pool(name="dp", bufs=4, space="DRAM") as dp,
            tc.tile_pool(name="pp", bufs=2, space="PSUM") as pp,
            tc.tile_pool(name="sp", bufs=2, space="PSUM") as sp,
            tc.tile_pool(name="vp", bufs=2, space="PSUM") as vp,
        ):
            wq_s = persist.tile([128, NCT, FH], F32R, tag="wq")
            wk_s = persist.tile([128, NCT, FH], F32R, tag="wk")
            wv_s = persist.tile([128, NCT, FH], F32R, tag="wv")
            wo_s = persist.tile([128, FT, C], F32R, tag="wo")
            wk_src = wk_h.ap().rearrange("(c p) f -> p c f", p=128)
            wv_src = wv_h.ap().rearrange("(c p) f -> p c f", p=128)
            xt0_pre = xp.tile([128, NCT, CH], F32R, tag="xt")
            xt0_src = xT[:, 0:CH].rearrange("(c p) t -> p c t", p=128)
            for cc in range(0, NCT, 2):
                nc.sync.dma_start(out=wk_s[:, cc:cc + 2, :], in_=wk_src[:, cc:cc + 2, :])
                nc.sync.dma_start(out=xt0_pre[:, cc:cc + 2, :], in_=xt0_src[:, cc:cc + 2, :])
            nc.sync.dma_start(out=wv_s[:, 0:4, :], in_=wv_src[:, 0:4, :])
            nc.sync.dma_start(out=wv_s[:, 4:8, :], in_=wv_src[:, 4:8, :])

            # causal 0/1 triangle: m0[p, f] = 1 iff f >= p. Every diagonal
            # 128-tile sees this same pattern in its own 128-column window.
            m0 = persist.tile([128, 128], BF16, tag="m0")
            nc.gpsimd.memset(m0, 1.0)
            nc.gpsimd.affine_select(
                out=m0, in_=m0, compare_op=mybir.AluOpType.is_ge, fill=0.0,
                base=0, pattern=[[1, 128]], channel_multiplier=-1,
            )

            kT_s = persist.tile([128, FT, T], F32R, tag="kT")
            v_s = persist.tile([128, 16, 8, 65], BF16, tag="vs")
            nc.gpsimd.memset(v_s[:, :, :, 64:65], 1.0)

            # ---- phase A: k^T and v projections, streamed over t-chunks ----
            SENT = object()
            for _rep in range(reps):
              for q in range(NQ):
                if q == 0 and _rep == 0:
                    xt = xt0_pre
                else:
                    xt = xp.tile([128, NCT, CH], F32R, tag="xt")
                    xt_src = xT[:, q * CH:(q + 1) * CH].rearrange("(c p) t -> p c t", p=128)
                    nc.sync.dma_start(out=xt[:, 0:4, :], in_=xt_src[:, 0:4, :])
                    nc.sync.dma_start(out=xt[:, 4:8, :], in_=xt_src[:, 4:8, :])
                for j in range(FT):
                    ps = pp.tile([128, 512], F32, tag="pp")
                    for c in range(NCT):
                        nc.tensor.matmul(
                            ps, wk_s[:, c, j * 128:(j + 1) * 128], xt[:, c, :],
                            start=(c == 0), stop=(c == NCT - 1),
                        )
                    nc.scalar.copy(out=kT_s[:, j, q * CH:(q + 1) * CH], in_=ps)
                for tt in range(4):
                    i = q * 4 + tt
                    ps = pp.tile([128, 512], F32, tag="pp")
                    for c in range(NCT):
                        nc.tensor.matmul(
                            ps, xt[:, c, tt * 128:(tt + 1) * 128], wv_s[:, c, :],
                            start=(c == 0), stop=(c == NCT - 1),
                        )
                    nc.vector.tensor_copy(
                        out=v_s[:, i, :, 0:64], in_=ps.rearrange("p (h d) -> p h d", h=8)
                    )
                if q == NQ - 1 and _rep == 0:
                    # q/o weights are first needed in phase B; keep them off the
                    # startup critical path
                    nc.sync.dma_start(out=wq_s, in_=wq_h.ap().rearrange("(c p) f -> p c f", p=128))
                    nc.sync.dma_start(out=wo_s, in_=wo_h.ap().rearrange("(i p) f -> p i f", p=128))

              # ---- phase B: per tq chunk: attention, with next chunk's q^T
              # projection and previous chunk's o_proj matmuls interleaved
              # into the exp-paced inner loop so the PE never starves ----
              def load_xt(lq):
                  t = xp.tile([128, NCT, CH], F32R, tag="xt")
                  src = xT[:, lq * CH:(lq + 1) * CH].rearrange("(c p) t -> p c t", p=128)
                  nc.sync.dma_start(out=t[:, 0:4, :], in_=src[:, 0:4, :])
                  nc.sync.dma_start(out=t[:, 4:8, :], in_=src[:, 4:8, :])
                  return t

              def qproj_steps(qT_t, xt_t):
                  for jj in range(FT):
                      ps = pp.tile([128, 512], F32, tag="pp")
                      for c in range(NCT):
                          nc.tensor.matmul(
                              ps, wq_s[:, c, jj * 128:(jj + 1) * 128], xt_t[:, c, :],
                              start=(c == 0), stop=(c == NCT - 1), skip_group_check=True,
                          )
                          yield
                      nc.vector.tensor_copy(out=qT_t[:, jj, :], in_=ps)
                      yield

              def oproj_steps(oq, at):
                  for n in range(2):
                      for mt in range(4):
                          po = pp.tile([128, 512], F32, tag="pp")
                          for i in range(FT):
                              nc.tensor.matmul(
                                  po, at[:, i, mt * 128:(mt + 1) * 128],
                                  wo_s[:, i, n * 512:(n + 1) * 512],
                                  start=(i == 0), stop=(i == FT - 1), skip_group_check=True,
                              )
                              yield
                          ot = opool.tile([128, 512], F32, tag="ot")
                          nc.vector.tensor_copy(out=ot, in_=po)
                          nc.sync.dma_start(
                              out=out_ap[oq * CH + mt * 128: oq * CH + (mt + 1) * 128,
                                         n * 512:(n + 1) * 512],
                              in_=ot,
                          )
                          yield

              def chain(*gens):
                  for gg in gens:
                      yield from gg

              prev = None
              xt_b = load_xt(0)
              qT_b = qp.tile([128, FT, CH], F32R, tag="qT")
              for _ in qproj_steps(qT_b, xt_b):
                  pass
              for q in range(NQ):
                  qT = qT_b
                  gens = []
                  n_steps = 0
                  if prev is not None:
                      gens.append(oproj_steps(*prev))
                      n_steps += 40
                  if q + 1 < NQ:
                      xt_b = load_xt(q + 1)
                      qT_b = qp.tile([128, FT, CH], F32R, tag="qT")
                      gens.append(qproj_steps(qT_b, xt_b))
                      n_steps += 36
                  stream = chain(*gens)
                  nkt = 4 * q + 4
                  # delay the stream when it starts with qproj (chunk 0): its
                  # xt DMA was only just issued
                  lead = 4 if prev is None else 0
                  total_kt = nkt * FT
                  emitted = 0
                  done_kt = 0
                  attnT = apool.tile([128, FT, CH], F32R, tag="attnT")
                  for j in range(FT):
                      pvA = vp.tile([65, 512], F32, tag="pv")
                      pvB = vp.tile([65, 512], F32, tag="pv")
                      def emit_pv(kkt, cc0, ptile):
                          nc.tensor.matmul(
                              pvA[:, cc0:512], v_s[:, kkt, 2 * j, :], ptile[:, 0, cc0:512],
                              start=(kkt == 0), stop=(kkt == nkt - 1), skip_group_check=True,
                          )
                          nc.tensor.matmul(
                              pvB[:, cc0:512], v_s[:, kkt, 2 * j + 1, :], ptile[:, 1, cc0:512],
                              start=(kkt == 0), stop=(kkt == nkt - 1), skip_group_check=True,
                          )
                      pend = None
                      for kt in range(nkt):
                          # diagonal tiles only produce valid scores for
                          # tq-chunk columns >= c0 = 128*(kt-4q); slice S, exp,
                          # mask and PV down to that window
                          rr = kt - 4 * q
                          c0 = 128 * rr if rr >= 0 else 0
                          s_ps = sp.tile([128, 1024], F32, tag="s")
                          nc.tensor.matmul(
                              s_ps[:, c0:512], kT_s[0:64, j, kt * 128:(kt + 1) * 128],
                              qT[0:64, j, c0:512], start=True, stop=True, tile_position=(0, 0),
                          )
                          nc.tensor.matmul(
                              s_ps[:, 512 + c0:1024], kT_s[64:128, j, kt * 128:(kt + 1) * 128],
                              qT[64:128, j, c0:512], start=True, stop=True, tile_position=(64, 0),
                          )
                          pt = ptp.tile([128, 2, 512], BF16, tag="pt")
                          s2 = s_ps.rearrange("p (h f) -> p h f", h=2)
                          if c0 == 0:
                              nc.scalar.activation(out=pt, in_=s2, func=EXP, scale=0.125)
                          else:
                              nc.scalar.activation(out=pt[:, :, c0:512], in_=s2[:, :, c0:512],
                                                   func=EXP, scale=0.125)
                          if rr >= 0:
                              nc.vector.tensor_mul(
                                  pt[:, :, c0:c0 + 128], pt[:, :, c0:c0 + 128],
                                  m0[:, None, :].broadcast_to([128, 2, 128]),
                              )
                          done_kt += 1
                          want = n_steps * max(0, done_kt - lead) // max(1, total_kt - lead)
                          while emitted < want:
                              if next(stream, SENT) is SENT:
                                  emitted = n_steps
                                  break
                              emitted += 1
                          if pend is not None:
                              emit_pv(*pend)
                          pend = (kt, c0, pt)
                      emit_pv(*pend)
                      # Move PV results out unnormalized (releases the PSUM
                      # accumulators fast), then normalize attnT in place once
                      # the reciprocal row returns from its DRAM broadcast
                      # round-trip (SBUF APs cannot have zero partition step;
                      # DRAM sources can).
                      rq = rp.tile([1, 1024], F32, tag="rec")
                      nc.vector.reciprocal(out=rq[0:1, 0:512], in_=pvA[64:65, :])
                      nc.vector.reciprocal(out=rq[0:1, 512:1024], in_=pvB[64:65, :])
                      nc.vector.tensor_copy(out=attnT[0:64, j, :], in_=pvA[0:64, :])
                      nc.vector.tensor_copy(out=attnT[64:128, j, :], in_=pvB[0:64, :])
                      rd = dp.tile([1, 1024], F32, tag="rd")
                      nc.sync.dma_start(out=rd, in_=rq)
                      rbc = rbp.tile([128, 512], F32, tag="rbc")
                      nc.sync.dma_start(out=rbc[0:64, :], in_=rd[0:1, 0:512].broadcast_to([64, 512]))
                      nc.sync.dma_start(out=rbc[64:128, :], in_=rd[0:1, 512:1024].broadcast_to([64, 512]))
                      nc.vector.tensor_mul(attnT[0:64, j, :], attnT[0:64, j, :], rbc[0:64, :])
                      nc.vector.tensor_mul(attnT[64:128, j, :], attnT[64:128, j, :], rbc[64:128, :])
                  for _ in stream:
                      pass
                  prev = (q, attnT)
              oproj_steps_tail = oproj_steps(*prev)
              for _ in oproj_steps_tail:
                  pass

    nc.compile()
    return nc


def _get_nc():
    if "nc" not in _CACHE:
        _CACHE["nc"] = _build()
    return _CACHE["nc"]


def make_in_maps(x, Wq, Wk, Wv, Wo):
    x = np.asarray(x, dtype=np.float32)
    Wq = np.asarray(Wq, dtype=np.float32)
    Wk = np.asarray(Wk, dtype=np.float32)
    Wv = np.asarray(Wv, dtype=np.float32)
    Wo = np.asarray(Wo, dtype=np.float32)
    in_maps = []
    for core in range(NCORES):
        b, g = core // 2, core % 2
        cols = slice(FH * g, FH * (g + 1))
        in_maps.append({
            "xT": np.ascontiguousarray(x[b].T),
            "wqT": np.ascontiguousarray(Wq.T[:, cols]),
            "wkT": np.ascontiguousarray(Wk.T[:, cols]),
            "wvT": np.ascontiguousarray(Wv.T[:, cols]),
            "woT": np.ascontiguousarray(Wo.T[cols, :]),
        })
    return in_maps


def gather_out(parts):
    return np.stack([parts[2 * b] + parts[2 * b + 1] for b in range(B)])


def kernel(x, Wq, Wk, Wv, Wo):
    from concourse.bass_utils import run_bass_kernel_spmd

    nc = _get_nc()
    in_maps = make_in_maps(x, Wq, Wk, Wv, Wo)
    res = run_bass_kernel_spmd(nc, in_maps, core_ids=list(range(NCORES)))
    return gather_out([res.results[c]["out"] for c in range(NCORES)])


# revision 33
# speedup vs baseline: 66.0558x; 1.0055x over previous
"""Causal self-attention (B=4, T=2048, C=1024, H=16, D=64) on 8 TRN2 NeuronCores.

Sharding: core c handles batch b=c//2 and head-group g=c%2 (8 of 16 heads,
i.e. 512 of 1024 feature columns). Each core:
  - projects q,k,v for its heads from x[b]  (contractions over C)
  - computes causal softmax(q k^T / sqrt(d)) v for its 8 heads
  - computes the partial o_proj  attn_out[:, cols] @ Wo[:, cols].T  -> [T, C]
Host sums the two head-group partials per batch and stacks batches.

Device layouts (host pre-transposes so every matmul contracts over the
partition dim with unit-stride DMAs):
  xT  [C, T]    = x[b].T
  wqT/wkT/wvT [C, 512] = W.T[:, cols]
  woT [512, C] = Wo.T[cols, :]
Scores are computed transposed (S^T[tk, tq] per head) so the PV matmul needs
no transpose; softmax denominators come from a ones-column appended to V
(PV matmul has M=65, row 64 = sum of weights). Two heads are packed per
128-partition tile; their K=64 QK matmuls run row-packed at
tile_position (0,0)/(64,0), and both heads share one [128,1024] exp.
"""

import numpy as np

B, T, C, H, D = 4, 2048, 1024, 16, 64
NCORES = 8
FH = 512          # features per core = 8 heads
NCT = C // 128    # 8 contraction tiles
FT = 4            # head-pair tiles (8 heads / 2)
NQ = 4            # tq chunks of 512
CH = 512          # tq chunk width

_CACHE = {}


def _build(reps=1):
    import concourse.bass as bass  # noqa: F401
    import concourse.mybir as mybir
    from concourse import bacc
    from concourse.tile import TileContext

    F32 = mybir.dt.float32
    F32R = mybir.dt.float32r
    BF16 = mybir.dt.bfloat16

    EXP = mybir.ActivationFunctionType.Exp

    nc = bacc.Bacc("TRN2", target_bir_lowering=False, debug=False, num_devices=NCORES)
    xT_h = nc.dram_tensor("xT", [C, T], F32R, kind="ExternalInput")
    wq_h = nc.dram_tensor("wqT", [C, FH], F32R, kind="ExternalInput")
    wk_h = nc.dram_tensor("wkT", [C, FH], F32R, kind="ExternalInput")
    wv_h = nc.dram_tensor("wvT", [C, FH], F32R, kind="ExternalInput")
    wo_h = nc.dram_tensor("woT", [FH, C], F32R, kind="ExternalInput")
    out_h = nc.dram_tensor("out", [T, C], F32, kind="ExternalOutput")
    xT = xT_h.ap()
    out_ap = out_h.ap()

    with TileContext(nc) as tc:
        with (
            tc.tile_pool(name="persist", bufs=1) as persist,
            tc.tile_pool(name="xp", bufs=2) as xp,
            tc.tile_pool(name="qp", bufs=2) as qp,
            tc.tile_pool(name="ptp", bufs=2) as ptp,
            tc.tile_pool(name="apl", bufs=2) as apool,
            tc.tile_pool(name="opl", bufs=2) as opool,
            tc.tile_pool(name="rp", bufs=2) as rp,
            tc.tile_pool(name="rbp", bufs=2) as rbp,
            tc.tile_pool(name="dp", bufs=4, space="DRAM") as dp,
            tc.tile_pool(name="pp", bufs=2, space="PSUM") as pp,
            tc.tile_pool(name="sp", bufs=2, space="PSUM") as sp,
            tc.tile_pool(name="vp", bufs=2, space="PSUM") as vp,
        ):
            wq_s = persist.tile([128, NCT, FH], F32R, tag="wq")
            wk_s = persist.tile([128, NCT, FH], F32R, tag="wk")
            wv_s = persist.tile([128, NCT, FH], F32R, tag="wv")
            wo_s = persist.tile([128, FT, C], F32R, tag="wo")
            wk_src = wk_h.ap().rearrange("(c p) f -> p c f", p=128)
            wv_src = wv_h.ap().rearrange("(c p) f -> p c f", p=128)
            xt0_pre = xp.tile([128, NCT, CH], F32R, tag="xt")
            xt0_src = xT[:, 0:CH].rearrange("(c p) t -> p c t", p=128)
            for cc in range(0, NCT, 2):
                nc.sync.dma_start(out=wk_s[:, cc:cc + 2, :], in_=wk_src[:, cc:cc + 2, :])
                nc.sync.dma_start(out=xt0_pre[:, cc:cc + 2, :], in_=xt0_src[:, cc:cc + 2, :])
            nc.sync.dma_start(out=wv_s[:, 0:4, :], in_=wv_src[:, 0:4, :])
            nc.sync.dma_start(out=wv_s[:, 4:8, :], in_=wv_src[:, 4:8, :])

            # causal 0/1 triangle: m0[p, f] = 1 iff f >= p. Every diagonal
            # 128-tile sees this same pattern in its own 128-column window.
            m0 = persist.tile([128, 128], BF16, tag="m0")
            nc.gpsimd.memset(m0, 1.0)
            nc.gpsimd.affine_select(
                out=m0, in_=m0, compare_op=mybir.AluOpType.is_ge, fill=0.0,
                base=0, pattern=[[1, 128]], channel_multiplier=-1,
            )

            kT_s = persist.tile([128, FT, T], F32R, tag="kT")
            v_s = persist.tile([128, 16, 8, 65], BF16, tag="vs")
            nc.gpsimd.memset(v_s[:, :, :, 64:65], 1.0)

            # ---- phase A: k^T and v projections, streamed over t-chunks ----
            SENT = object()
            for _rep in range(reps):
              for q in range(NQ):
                if q == 0 and _rep == 0:
                    xt = xt0_pre
                else:
                    xt = xp.tile([128, NCT, CH], F32R, tag="xt")
                    xt_src = xT[:, q * CH:(q + 1) * CH].rearrange("(c p) t -> p c t", p=128)
                    nc.sync.dma_start(out=xt[:, 0:4, :], in_=xt_src[:, 0:4, :])
                    nc.sync.dma_start(out=xt[:, 4:8, :], in_=xt_src[:, 4:8, :])
                for j in range(FT):
                    ps = pp.tile([128, 512], F32, tag="pp")
                    for c in range(NCT):
                        nc.tensor.matmul(
                            ps, wk_s[:, c, j * 128:(j + 1) * 128], xt[:, c, :],
                            start=(c == 0), stop=(c == NCT - 1),
                        )
                    nc.scalar.copy(out=kT_s[:, j, q * CH:(q + 1) * CH], in_=ps)
                for tt in range(4):
                    i = q * 4 + tt
                    ps = pp.tile([128, 512], F32, tag="pp")
                    for c in range(NCT):
                        nc.tensor.matmul(
                            ps, xt[:, c, tt * 128:(tt + 1) * 128], wv_s[:, c, :],
                            start=(c == 0), stop=(c == NCT - 1),
                        )
                    nc.vector.tensor_copy(
                        out=v_s[:, i, :, 0:64], in_=ps.rearrange("p (h d) -> p h d", h=8)
                    )
                if q == NQ - 1 and _rep == 0:
                    # q/o weights are first needed in phase B; keep them off the
                    # startup critical path
                    nc.sync.dma_start(out=wq_s, in_=wq_h.ap().rearrange("(c p) f -> p c f", p=128))
                    nc.sync.dma_start(out=wo_s, in_=wo_h.ap().rearrange("(i p) f -> p i f", p=128))

              # ---- phase B: per tq chunk: attention, with next chunk's q^T
              # projection and previous chunk's o_proj matmuls interleaved
              # into the exp-paced inner loop so the PE never starves ----
              def load_xt(lq):
                  t = xp.tile([128, NCT, CH], F32R, tag="xt")
                  src = xT[:, lq * CH:(lq + 1) * CH].rearrange("(c p) t -> p c t", p=128)
                  nc.sync.dma_start(out=t[:, 0:4, :], in_=src[:, 0:4, :])
                  nc.sync.dma_start(out=t[:, 4:8, :], in_=src[:, 4:8, :])
                  return t

              def qproj_steps(qT_t, xt_t):
                  for jj in range(FT):
                      ps = pp.tile([128, 512], F32, tag="pp")
                      for c in range(NCT):
                          nc.tensor.matmul(
                              ps, wq_s[:, c, jj * 128:(jj + 1) * 128], xt_t[:, c, :],
                              start=(c == 0), stop=(c == NCT - 1), skip_group_check=True,
                          )
                          yield
                      nc.vector.tensor_copy(out=qT_t[:, jj, :], in_=ps)
                      yield

              def oproj_steps(oq, at):
                  for n in range(2):
                      for mt in range(4):
                          po = pp.tile([128, 512], F32, tag="pp")
                          for i in range(FT):
                              nc.tensor.matmul(
                                  po, at[:, i, mt * 128:(mt + 1) * 128],
                                  wo_s[:, i, n * 512:(n + 1) * 512],
                                  start=(i == 0), stop=(i == FT - 1), skip_group_check=True,
                              )
                              yield
                          ot = opool.tile([128, 512], F32, tag="ot")
                          nc.vector.tensor_copy(out=ot, in_=po)
                          nc.sync.dma_start(
                              out=out_ap[oq * CH + mt * 128: oq * CH + (mt + 1) * 128,
                                         n * 512:(n + 1) * 512],
                              in_=ot,
                          )
                          yield

              def chain(*gens):
                  for gg in gens:
                      yield from gg

              prev = None
              xt_b = load_xt(0)
              qT_b = qp.tile([128, FT, CH], F32R, tag="qT")
              for _ in qproj_steps(qT_b, xt_b):
                  pass
              for q in range(NQ):
                  qT = qT_b
                  gens = []
                  n_steps = 0
                  if prev is not None:
                      gens.append(oproj_steps(*prev))
                      n_steps += 40
                  if q + 1 < NQ:
                      xt_b = load_xt(q + 1)
                      qT_b = qp.tile([128, FT, CH], F32R, tag="qT")
                      gens.append(qproj_steps(qT_b, xt_b))
                      n_steps += 36
                  stream = chain(*gens)
                  nkt = 4 * q + 4
                  # delay the stream when it starts with qproj (chunk 0): its
                  # xt DMA was only just issued
                  lead = 4 if prev is None else 0
                  total_kt = nkt * FT
                  emitted = 0
                  done_kt = 0
                  attnT = apool.tile([128, FT, CH], F32R, tag="attnT")
                  for j in range(FT):
                      pvA = vp.tile([65, 512], F32, tag="pv")
                      pvB = vp.tile([65, 512], F32, tag="pv")
                      def emit_pv(kkt, cc0, ptile):
                          nc.tensor.matmul(
                              pvA[:, cc0:512], v_s[:, kkt, 2 * j, :], ptile[:, 0, cc0:512],
                              start=(kkt == 0), stop=(kkt == nkt - 1), skip_group_check=True,
                          )
                          nc.tensor.matmul(
                              pvB[:, cc0:512], v_s[:, kkt, 2 * j + 1, :], ptile[:, 1, cc0:512],
                              start=(kkt == 0), stop=(kkt == nkt - 1), skip_group_check=True,
                          )
                      pend = None
                      for kt in range(nkt):
                          # diagonal tiles only produce valid scores for
                          # tq-chunk columns >= c0 = 128*(kt-4q); slice S, exp,
                          # mask and PV down to that window
                          rr = kt - 4 * q
                          c0 = 128 * rr if rr >= 0 else 0
                          s_ps = sp.tile([128, 1024], F32, tag="s")
                          nc.tensor.matmul(
                              s_ps[:, c0:512], kT_s[0:64, j, kt * 128:(kt + 1) * 128],
                              qT[0:64, j, c0:512], start=True, stop=True, tile_position=(0, 0),
                          )
                          nc.tensor.matmul(
                              s_ps[:, 512 + c0:1024], kT_s[64:128, j, kt * 128:(kt + 1) * 128],
                              qT[64:128, j, c0:512], start=True, stop=True, tile_position=(64, 0),
                          )
                          pt = ptp.tile([128, 2, 512], BF16, tag="pt")
                          s2 = s_ps.rearrange("p (h f) -> p h f", h=2)
                          if c0 == 0:
                              nc.scalar.activation(out=pt, in_=s2, func=EXP, scale=0.125)
                          else:
                              nc.scalar.activation(out=pt[:, :, c0:512], in_=s2[:, :, c0:512],
                                                   func=EXP, scale=0.125)
                          if rr >= 0:
                              nc.vector.tensor_mul(
                                  pt[:, :, c0:c0 + 128], pt[:, :, c0:c0 + 128],
                                  m0[:, None, :].broadcast_to([128, 2, 128]),
                              )
                          done_kt += 1
                          want = n_steps * max(0, done_kt - lead) // max(1, total_kt - lead)
                          while emitted < want:
                              if next(stream, SENT) is SENT:
                                  emitted = n_steps
                                  break
                              emitted += 1
                          if pend is not None:
                              emit_pv(*pend)
                          pend = (kt, c0, pt)
                      emit_pv(*pend)
                      # Move PV results out unnormalized (releases the PSUM
                      # accumulators fast), then normalize attnT in place once
                      # the reciprocal row returns from its DRAM broadcast
                      # round-trip (SBUF APs cannot have zero partition step;
                      # DRAM sources can).
                      rq = rp.tile([1, 1024], F32, tag="rec")
                      nc.vector.reciprocal(out=rq[0:1, 0:512], in_=pvA[64:65, :])
                      nc.vector.reciprocal(out=rq[0:1, 512:1024], in_=pvB[64:65, :])
                      nc.vector.tensor_copy(out=attnT[0:64, j, :], in_=pvA[0:64, :])
                      nc.vector.tensor_copy(out=attnT[64:128, j, :], in_=pvB[0:64, :])
                      rd = dp.tile([1, 1024], F32, tag="rd")
                      nc.sync.dma_start(out=rd, in_=rq)
                      rbc = rbp.tile([128, 512], F32, tag="rbc")
                      nc.sync.dma_start(out=rbc[0:64, :], in_=rd[0:1, 0:512].broadcast_to([64, 512]))
                      nc.sync.dma_start(out=rbc[64:128, :], in_=rd[0:1, 512:1024].broadcast_to([64, 512]))
                      nc.vector.tensor_mul(attnT[0:64, j, :], attnT[0:64, j, :], rbc[0:64, :])
                      nc.vector.tensor_mul(attnT[64:128, j, :], attnT[64:128, j, :], rbc[64:128, :])
                  for _ in stream:
                      pass
                  prev = (q, attnT)
              # tail o_proj: all groups' pair-0..2 accumulations first (dense
              # PE work that hides the last pair's reciprocal round-trip),
              # then the pair-3 close-outs. The score/PV PSUM banks are free
              # at the tail, so 6 accumulation groups stay open at once.
              oq, at = prev
              groups = [(n, mt) for n in range(2) for mt in range(4)]
              open_tiles = []
              for gi, (n, mt) in enumerate(groups[:6]):
                  if gi < 2:
                      po = pp.tile([128, 512], F32, tag="pp")
                  elif gi < 4:
                      po = sp.tile([128, 512], F32, tag="s")
                  else:
                      po = vp.tile([128, 512], F32, tag="pv")
                  open_tiles.append(po)
                  for i in range(FT - 1):
                      nc.tensor.matmul(
                          po, at[:, i, mt * 128:(mt + 1) * 128],
                          wo_s[:, i, n * 512:(n + 1) * 512],
                          start=(i == 0), stop=False, skip_group_check=True,
                      )
              for gi, (n, mt) in enumerate(groups[:6]):
                  po = open_tiles[gi]
                  nc.tensor.matmul(
                      po, at[:, FT - 1, mt * 128:(mt + 1) * 128],
                      wo_s[:, FT - 1, n * 512:(n + 1) * 512],
                      start=False, stop=True, skip_group_check=True,
                  )
                  ot = opool.tile([128, 512], F32, tag="ot")
                  nc.vector.tensor_copy(out=ot, in_=po)
                  nc.sync.dma_start(
                      out=out_ap[oq * CH + mt * 128: oq * CH + (mt + 1) * 128,
                                 n * 512:(n + 1) * 512],
                      in_=ot,
                  )
              for (n, mt) in groups[6:]:
                  po = pp.tile([128, 512], F32, tag="pp")
                  for i in range(FT):
                      nc.tensor.matmul(
                          po, at[:, i, mt * 128:(mt + 1) * 128],
                          wo_s[:, i, n * 512:(n + 1) * 512],
                          start=(i == 0), stop=(i == FT - 1), skip_group_check=True,
                      )
                  ot = opool.tile([128, 512], F32, tag="ot")
                  nc.vector.tensor_copy(out=ot, in_=po)
                  nc.sync.dma_start(
                      out=out_ap[oq * CH + mt * 128: oq * CH + (mt + 1) * 128,
                                 n * 512:(n + 1) * 512],
                      in_=ot,
                  )

    nc.compile()
    return nc


def _get_nc():
    if "nc" not in _CACHE:
        _CACHE["nc"] = _build()
    return _CACHE["nc"]


def make_in_maps(x, Wq, Wk, Wv, Wo):
    x = np.asarray(x, dtype=np.float32)
    Wq = np.asarray(Wq, dtype=np.float32)
    Wk = np.asarray(Wk, dtype=np.float32)
    Wv = np.asarray(Wv, dtype=np.float32)
    Wo = np.asarray(Wo, dtype=np.float32)
    in_maps = []
    for core in range(NCORES):
        b, g = core // 2, core % 2
        cols = slice(FH * g, FH * (g + 1))
        in_maps.append({
            "xT": np.ascontiguousarray(x[b].T),
            "wqT": np.ascontiguousarray(Wq.T[:, cols]),
            "wkT": np.ascontiguousarray(Wk.T[:, cols]),
            "wvT": np.ascontiguousarray(Wv.T[:, cols]),
            "woT": np.ascontiguousarray(Wo.T[cols, :]),
        })
    return in_maps


def gather_out(parts):
    return np.stack([parts[2 * b] + parts[2 * b + 1] for b in range(B)])


def kernel(x, Wq, Wk, Wv, Wo):
    from concourse.bass_utils import run_bass_kernel_spmd

    nc = _get_nc()
    in_maps = make_in_maps(x, Wq, Wk, Wv, Wo)
    res = run_bass_kernel_spmd(nc, in_maps, core_ids=list(range(NCORES)))
    return gather_out([res.results[c]["out"] for c in range(NCORES)])
